# revision 16
# baseline (speedup 1.0000x reference)
"""Causal linear attention (ELU+1 feature map) on 8 trn2 NeuronCores.

Sharding: core i handles batch b=i//2, sequence half h=i%2 (T=2048 -> 1024
tokens/core).  Second-half cores recompute the first half's running state
from k/v of the first half; first-half cores get zeroed aux inputs.

v2: fused phi via scalar_tensor_tensor, 4-way parity state, merged pair
ops, 3 parallel HWDGE queues, on-chip mask/ident, bf16 output.

Math per core (chunk C=128, 8 own chunks + 8 "pre" state-only chunks):
  phi(y) = min(exp(y+b), 1) + relu(y+b)   (== ELU(y+b)+1 exactly)
         = stt(y, max, -b) + [min(exp(y+b),1) + b]
"""

import numpy as np

B, T, D, DV = 4, 2048, 128, 128
H = T // 2          # tokens per core
C = 128             # chunk
NCH = H // C        # chunks per half
NCORES = 8
VW = DV + 1
NPAR = 2

# bf16 pack columns: [WTb | bias | kTp | vp | kT | qT | v]
OFF_WTB = 0
OFF_BIAS = OFF_WTB + D
OFF_KTP = OFF_BIAS + 1
OFF_VP = OFF_KTP + H
OFF_KT = OFF_VP + NCH * VW
OFF_QT = OFF_KT + H
OFF_V = OFF_QT + H
B16_COLS = OFF_V + NCH * VW

CFG = {
    "min_eng": ("dve", "dve", "dve", "pool", "pool", "pool"),  # pre0 pre1 K0 Q0 K1 Q1
    "snap_eng": ("act", "act", "act", "act", "act", "act", "act"),  # c=1..7
    "snap0_eng": "act",
    "scale_eng": ("dve", "act", "dve", "act", "dve", "act", "dve", "act"),
    "ktok_eng": ("act", "dve", "act", "dve"),   # 4 pair copies
    "mask_eng": ("dve", "dve", "dve", "dve"),   # 4 pair masks
    "out_q": ("sync", "act", "sync", "act"),
}

_cache = {}


def _build(cfg=None):
    import concourse.bacc as bacc
    import concourse.tile as tile
    from concourse import mybir
    from bass_rust import add_dep_helper

    cfg = dict(CFG, **(cfg or {}))
    F32 = mybir.dt.float32
    BF16 = mybir.dt.bfloat16
    AF = mybir.ActivationFunctionType
    OP = mybir.AluOpType

    nc = bacc.Bacc(None, target_bir_lowering=False, debug=False,
                   num_devices=NCORES)

    bin_ = nc.declare_dram_parameter("bin", [D, B16_COLS], BF16, isOutput=False)
    btile = nc.declare_dram_parameter("btile", [1, H], BF16, isOutput=False)
    out = nc.declare_dram_parameter("out", [C, NCH * DV], BF16, isOutput=True)

    def eng(name):
        return {"dve": nc.vector, "act": nc.scalar, "pool": nc.gpsimd,
                "sync": nc.sync}[name]

    def copy_on(which, dst, src):
        if which == "act":
            nc.scalar.activation(dst, src, AF.Copy)
        elif which == "pool":
            nc.gpsimd.tensor_copy(dst, src)
        else:
            nc.vector.tensor_copy(dst, src)

    with tile.TileContext(nc) as tc:
        with (
            tc.tile_pool(name="cst", bufs=1) as cst,
            tc.tile_pool(name="io", bufs=1) as io,
            tc.tile_pool(name="phi", bufs=1) as phip,
            tc.tile_pool(name="etmp", bufs=3) as etp,
            tc.tile_pool(name="am", bufs=4) as amp,
            tc.tile_pool(name="wrk", bufs=3) as wrk,
            tc.tile_pool(name="ps_y", bufs=3, space="PSUM") as ps_y,
            tc.tile_pool(name="ps_s", bufs=1, space="PSUM") as ps_s,
            tc.tile_pool(name="ps_w", bufs=3, space="PSUM") as ps_w,
        ):
            # ---- warm the ACT exp table while DMAs run ----
            s_warm = cst.tile([D, 1], F32)
            nc.vector.memset(s_warm, 0.0)
            s_warm2 = cst.tile([D, 1], BF16)
            nc.scalar.activation(s_warm2, s_warm, AF.Exp)

            # ---- input DMAs: 3 hwdge queues, need-ordered ----
            s_b16 = io.tile([D, B16_COLS], BF16)
            s_btile = cst.tile([1, H], BF16)
            hh = H // 2
            vh = OFF_V + (NCH // 2) * VW
            pieces_sync = [(0, OFF_KTP + hh),                 # WTb+bias+kTp0
                           (OFF_KTP + hh, OFF_VP),            # kTp 2nd
                           (OFF_KT + hh, OFF_QT),             # kT 2nd
                           (OFF_V, vh),                       # v 1st
                           (vh, B16_COLS)]                    # v 2nd
            pieces_scalar = [(OFF_KT, OFF_KT + hh),           # kT 1st
                             (OFF_QT, OFF_QT + hh),           # qT 1st
                             (OFF_QT + hh, OFF_V)]            # qT 2nd
            pieces_pool = [(OFF_VP, OFF_KT)]                  # vp
            for a, b in pieces_scalar:
                nc.scalar.dma_start(out=s_b16[:, a:b], in_=bin_[:, a:b])
            first_sync = True
            for a, b in pieces_sync:
                nc.sync.dma_start(out=s_b16[:, a:b], in_=bin_[:, a:b])
                if first_sync:
                    nc.sync.dma_start(out=s_btile, in_=btile[:, :])
                    first_sync = False
            for a, b in pieces_pool:
                nc.gpsimd.dma_start(out=s_b16[:, a:b], in_=bin_[:, a:b])

            # ---- on-chip constants ----
            s_ones1 = cst.tile([1, C], BF16)
            nc.vector.memset(s_ones1, 1.0)
            s_ones = cst.tile([D, 2 * C], BF16)
            nc.vector.memset(s_ones, 1.0)
            s_mask2 = cst.tile([D, 2 * C], BF16)    # [mask|mask], keep f>=p
            nc.gpsimd.affine_select(out=s_mask2, in_=s_ones,
                                    pattern=[[0, 2], [1, C]],
                                    compare_op=OP.is_ge, fill=0.0,
                                    base=0, channel_multiplier=-1)
            s_ident = cst.tile([D, C], BF16)
            nc.gpsimd.affine_select(out=s_ident, in_=s_ones[:, 0:C],
                                    pattern=[[1, C]],
                                    compare_op=OP.is_equal, fill=0.0,
                                    base=0, channel_multiplier=-1)

            s_bias = s_b16[:, OFF_BIAS:OFF_BIAS + 1]
            s_bias32 = cst.tile([D, 1], F32)
            nc.vector.tensor_copy(s_bias32, s_bias)
            s_negb32 = cst.tile([D, 1], F32)
            nc.vector.tensor_scalar_mul(s_negb32, s_bias32, -1.0)
            sWTb = s_b16[:, OFF_WTB:OFF_WTB + D]

            def vsl(c):
                return s_b16[:, OFF_V + VW * c:OFF_V + VW * (c + 1)]

            def vpsl(c):
                return s_b16[:, OFF_VP + VW * c:OFF_VP + VW * (c + 1)]

            # parity state PSUMs: one bank per parity (PSUM allows only one
            # open accumulation group per bank)
            Sp = []
            for i in range(NPAR):
                s_par = ps_s.tile([D, VW], F32, tag=f"s{i}")
                Sp.append(s_par)

            def s_region(p):
                return Sp[p]

            started = [False] * NPAR
            s_last = [None] * NPAR

            def u_mm(p, ktok_sl, v_sl, stop):
                mm = nc.tensor.matmul(s_region(p), ktok_sl, v_sl,
                                      start=(not started[p]), stop=stop,
                                      skip_group_check=True)
                if started[p]:
                    add_dep_helper(mm.ins, s_last[p].ins, sync=False,
                                   reason="psum group order")
                s_last[p] = mm
                started[p] = True
                return mm

            # snapshot SBUF: [s0|s1|s2|s3] each VW cols
            snapbuf = phip.tile([D, NPAR * VW], BF16)

            min_i = [0]

            def phi_min(dst, y, em, scalar=0.0):
                nc.vector.scalar_tensor_tensor(out=dst, in0=y, scalar=scalar,
                                               in1=em, op0=OP.max, op1=OP.add)

            # ---- PRE path: token-major phi(kTp) -> U_pre into parities ----
            phi_t = phip.tile([C, H], BF16)
            pre_ys = [None, None]

            def pre_y(j):
                y = ps_y.tile([C, 512], F32, tag="y")
                pre_ys[j] = y
                prev = nc.tensor.matmul(y, s_ones1,
                                        s_btile[:, 512 * j:512 * (j + 1)],
                                        start=True, stop=False)
                for cc in range(4):
                    c = 4 * j + cc
                    mm = nc.tensor.matmul(
                        y[:, C * cc:C * (cc + 1)],
                        s_b16[:, OFF_KTP + C * c:OFF_KTP + C * (c + 1)],
                        sWTb, start=False, stop=(cc == 3))
                    add_dep_helper(mm.ins, prev.ins, sync=False,
                                   reason="psum group order")
                    prev = mm

            def pre_nl(j):
                y = pre_ys[j]
                e = etp.tile([C, 512], BF16, tag="e")
                nc.scalar.activation(e, y, AF.Exp)
                em = etp.tile([C, 512], BF16, tag="em")
                w = cfg["min_eng"][j]
                if w == "pool":
                    nc.gpsimd.tensor_scalar_min(em, e, 1.0)
                else:
                    nc.vector.tensor_scalar_min(em, e, 1.0)
                sl = slice(512 * j, 512 * (j + 1))
                phi_min(phi_t[:, sl], y, em)

            def pre_u(j):
                for cc in range(4):
                    c = 4 * j + cc
                    u_mm(c % NPAR, phi_t[:, C * c:C * (c + 1)], vpsl(c),
                         stop=False)

            # ---- feature-major phi slices ----
            QT = phip.tile([D, H], BF16)
            KT = phip.tile([D, H], BF16)

            slice_ys = {}

            def phi_y(key, off, j):
                y = ps_y.tile([D, 512], F32, tag="y")
                slice_ys[key] = y
                nc.tensor.matmul(y, sWTb,
                                 s_b16[:, off + 512 * j:off + 512 * (j + 1)],
                                 start=True, stop=True)

            def phi_nl(key, dst, j):
                y = slice_ys[key]
                e = etp.tile([D, 512], BF16, tag="e")
                nc.scalar.activation(e, y, AF.Exp, bias=s_bias, scale=1.0)
                em = etp.tile([D, 512], BF16, tag="em")
                w = cfg["min_eng"][2 + min_i[0]]
                min_i[0] += 1
                if w == "pool":
                    nc.gpsimd.tensor_scalar(out=em, in0=e, scalar1=1.0,
                                            scalar2=s_bias32, op0=OP.min,
                                            op1=OP.add)
                else:
                    nc.vector.tensor_scalar(out=em, in0=e, scalar1=1.0,
                                            scalar2=s_bias32, op0=OP.min,
                                            op1=OP.add)
                phi_min(dst[:, 512 * j:512 * (j + 1)], y, em, s_negb32)

            ktok = phip.tile([C, H], BF16)

            def prep_k(pair):  # pair of chunks (2p, 2p+1): transpose+copy
                trp = ps_w.tile([C, 2 * C], BF16, tag="w")
                for i in range(2):
                    c = 2 * pair + i
                    nc.tensor.transpose(trp[:, C * i:C * (i + 1)],
                                        KT[:, C * c:C * (c + 1)], s_ident)
                copy_on(cfg["ktok_eng"][pair],
                        ktok[:, 2 * C * pair:2 * C * (pair + 1)], trp)

            Am = [None] * (NCH // 2)

            def prep_a(pair):  # A for chunks (2p, 2p+1) + mask
                A2 = ps_w.tile([C, 2 * C], F32, tag="w")
                for i in range(2):
                    c = 2 * pair + i
                    nc.tensor.matmul(A2[:, C * i:C * (i + 1)],
                                     KT[:, C * c:C * (c + 1)],
                                     QT[:, C * c:C * (c + 1)],
                                     start=True, stop=True)
                am2 = amp.tile([C, 2 * C], BF16, tag="am")
                Am[pair] = am2
                w = cfg["mask_eng"][pair]
                if w == "pool":
                    nc.gpsimd.tensor_tensor(out=am2, in0=A2, in1=s_mask2,
                                            op=OP.mult)
                else:
                    nc.vector.tensor_tensor(out=am2, in0=A2, in1=s_mask2,
                                            op=OP.mult)

            outstage = phip.tile([C, NCH * DV], BF16)

            def snap_pair(which, half):
                copy_on(which, snapbuf[:, VW * half:VW * (half + 1)],
                        Sp[half])

            def snap_one(which, p):
                copy_on(which, snapbuf[:, VW * p:VW * (p + 1)], s_region(p))

            def run_chunk(c):
                if c == 0:
                    w = cfg["snap0_eng"]
                    snap_pair(w, 0)
                    snap_pair(w, 1)
                else:
                    snap_one(cfg["snap_eng"][c - 1], (c - 1) % NPAR)

                O = ps_w.tile([C, VW], F32, tag="w")
                prev_o = nc.tensor.matmul(O, Am[c // 2][:, C * (c % 2):C * (c % 2 + 1)],
                                          vsl(c), start=True, stop=False)
                for p in range(NPAR):
                    mm = nc.tensor.matmul(O, QT[:, C * c:C * (c + 1)],
                                          snapbuf[:, VW * p:VW * (p + 1)],
                                          start=False, stop=(p == NPAR - 1))
                    add_dep_helper(mm.ins, prev_o.ins, sync=False,
                                   reason="psum group order")
                    prev_o = mm

                u_mm(c % NPAR, ktok[:, C * c:C * (c + 1)], vsl(c),
                     stop=(c >= NCH - NPAR))

                rec = wrk.tile([C, 1], F32, tag="rec")
                nc.vector.reciprocal(rec, O[:, DV:DV + 1])
                se = cfg["scale_eng"][c]
                if se == "act":
                    nc.scalar.activation(outstage[:, DV * c:DV * (c + 1)],
                                         O[:, 0:DV], AF.Copy, bias=0.0,
                                         scale=rec)
                else:
                    nc.vector.tensor_scalar_mul(
                        outstage[:, DV * c:DV * (c + 1)], O[:, 0:DV], rec)
                if c % 2 == 1:
                    q = cfg["out_q"][c // 2]
                    eng(q).dma_start(
                        out=out[:, DV * (c - 1):DV * (c + 1)],
                        in_=outstage[:, DV * (c - 1):DV * (c + 1)])

            # ---- schedule: all phi first (priority), then chunk stream ----
            pre_y(0)
            pre_y(1)
            phi_y("K0", OFF_KT, 0)
            pre_nl(0)
            pre_nl(1)
            phi_nl("K0", KT, 0)
            phi_y("Q0", OFF_QT, 0)
            phi_nl("Q0", QT, 0)
            phi_y("K1", OFF_KT, 1)
            phi_nl("K1", KT, 1)
            phi_y("Q1", OFF_QT, 1)
            phi_nl("Q1", QT, 1)
            pre_u(0)
            pre_u(1)
            prep_k(0)
            prep_k(1)
            prep_a(0)
            prep_a(1)
            prep_k(2)
            prep_k(3)
            prep_a(2)
            prep_a(3)
            for c in range(NCH):
                run_chunk(c)

    nc.compile()
    return nc


def _get_nc():
    if "nc" not in _cache:
        _cache["nc"] = _build()
    return _cache["nc"]


def _pack_inputs(q, k, v, W_phi, b_phi):
    import ml_dtypes
    bf16 = ml_dtypes.bfloat16

    WT = np.ascontiguousarray(W_phi.T)                    # [d, e]
    btile = np.tile(b_phi, NCH).reshape(1, H).astype(bf16)

    def aug(vh):  # [H, DV] -> [C, NCH*(DV+1)] partition-major with ones col
        a = np.concatenate([vh, np.ones((H, 1), np.float32)], axis=1)
        return a.reshape(NCH, C, VW).transpose(1, 0, 2).reshape(C, NCH * VW)

    zeros_vp = np.zeros((C, NCH * VW), np.float32)
    zeros_ktp = np.zeros((D, H), np.float32)

    in_maps = []
    for core in range(NCORES):
        b_idx, half = divmod(core, 2)
        sl = slice(half * H, (half + 1) * H)
        b16 = np.empty((D, B16_COLS), np.float32)
        b16[:, OFF_WTB:OFF_WTB + D] = WT
        b16[:, OFF_BIAS] = b_phi
        b16[:, OFF_QT:OFF_QT + H] = q[b_idx, sl].T
        b16[:, OFF_KT:OFF_KT + H] = k[b_idx, sl].T
        if half == 1:
            b16[:, OFF_KTP:OFF_KTP + H] = k[b_idx, 0:H].T
            b16[:, OFF_VP:OFF_VP + NCH * VW] = aug(v[b_idx, 0:H])
        else:
            b16[:, OFF_KTP:OFF_KTP + H] = zeros_ktp
            b16[:, OFF_VP:OFF_VP + NCH * VW] = zeros_vp
        b16[:, OFF_V:OFF_V + NCH * VW] = aug(v[b_idx, sl])
        in_maps.append({"bin": b16.astype(bf16), "btile": btile})
    return in_maps


def kernel(q, k, v, W_phi, b_phi):
    from concourse.bass_utils import run_bass_kernel_spmd

    q = np.asarray(q, np.float32)
    k = np.asarray(k, np.float32)
    v = np.asarray(v, np.float32)
    W_phi = np.asarray(W_phi, np.float32)
    b_phi = np.asarray(b_phi, np.float32)

    in_maps = _pack_inputs(q, k, v, W_phi, b_phi)
    nc = _get_nc()
    res = run_bass_kernel_spmd(nc, in_maps, list(range(NCORES)))

    out = np.empty((B, T, DV), np.float32)
    for core in range(NCORES):
        b_idx, half = divmod(core, 2)
        o = np.asarray(res.results[core]["out"], np.float32)  # [C, NCH*DV]
        o = o.reshape(C, NCH, DV).transpose(1, 0, 2).reshape(H, DV)
        out[b_idx, half * H:(half + 1) * H] = o
    return out
